# revision 15
# baseline (speedup 1.0000x reference)
"""GumbelSparseAttention kernel for 8 Trainium2 NeuronCores (v4).

Reference semantics (B=1, L=2048, E=1024, H=16, d=64, TAU=0.1):
  scores = (q @ k^T) * d**-0.5                     per head   [L, L]
  logits = q.mean(-1) @ w_gumbel^T + b_gumbel      per head   [L]
  mask   = one_hot(argmax(logits + gumbel(u)))  (straight-through -> exact 0/1)
  out[l] = softmax(scores[l] * mask[l]) @ v
The mask is one-hot over the *query* axis, so only one row per head gets real
attention; every other row's scores are exactly 0 -> uniform softmax -> row =
mean(v).  Per head the kernel computes: the logits argmax, one attention row,
and the v column means.

Speed structure:
  - w/k/v stream in pre-transposed/pre-arranged (host layout-only) and in
    bf16, halving both DMA bytes and PE streaming time (PSUM accum is fp32).
  - gumbel g/8 is precomputed on-scalar and folded into the partial logits
    by vector/pool adds that overlap the PE matmuls, pre-ReduceScatter.
  - DMA issue is spread across engine descriptor rings so the lpart transfer
    (which gates the collective) never queues behind bulk loads.
  - output is stored transposed; the v-mean fill is one broadcast DMA and the
    two real attention rows are emitted separately (host places them).

Sharding (8 cores): w_gumbel split by columns (contraction j) -> partial
logits [16, L] per core -> ReduceScatter(add) -> each core owns 2 heads.
"""

import sys

sys.path.insert(0, "/opt/trn_rl_repo")

import numpy as np  # noqa: E402
import ml_dtypes  # noqa: E402
import concourse.bass as bass  # noqa: E402
import concourse.mybir as mybir  # noqa: E402
import concourse.tile as tile  # noqa: E402
from concourse.tile import TileContext  # noqa: E402
from concourse.masks import make_identity  # noqa: E402
from concourse.vector_clock import ScopedClock, VectorClock  # noqa: E402

F32 = mybir.dt.float32
BF16 = mybir.dt.bfloat16
I32 = mybir.dt.int32
U32 = mybir.dt.uint32
BF16NP = ml_dtypes.bfloat16

N_CORES = 8
L = 2048
E = 1024
H = 16
D = 64
HPC = H // N_CORES          # heads per core = 2
JC = L // N_CORES           # w_gumbel column chunk (contraction) = 256
QC = L // N_CORES           # q row chunk = 256
NCH = L // 128              # 16 position chunks
SCALE = D ** -0.5           # 0.125
VW = 132                    # vtx stride: 128 v cols + 1 ones col + 3 pad
AF = mybir.ActivationFunctionType
ALU = mybir.AluOpType


# ---------------------------------------------------------------------------
# Workarounds for this toolchain's walrus: it rejects instructions carrying
# more than ~2 semaphore waits, including the Tile tail drain.
# ---------------------------------------------------------------------------

def _patched_drain_and_barrier(self, tick_clock, wait_clock):
    gc = tick_clock.global_clock
    n = len(gc)
    for i in range(n):
        t = gc[i]
        if t > 0:
            vec = [0] * n
            vec[i] = t
            nop = self.nc.sync.nop()
            wait_clock.add_sem_waits(nop.ins, ScopedClock({None: VectorClock(vec)}))
    self.nc.sync.drain()  # waits already handled by the NOP cascade above
    self.nc.all_engine_barrier()
    assert self.sems is not None
    popped = self.nc._tile_sem_poison_stack.pop()
    assert popped is self._sem_poison
    self.nc.clear_and_free_semaphores(list(self.sems.allocated().values()))
    self.nc.all_engine_barrier()


tile.TileContext._drain_and_barrier = _patched_drain_and_barrier


def _split_excess_waits(nc, max_waits=1):
    nsplit = 0
    for fn in nc.m.functions:
        for blk in fn.blocks:
            insts = list(blk.instructions)
            new = []
            for ins in insts:
                si = ins.sync_info
                if si is not None and len(si.on_wait) > max_waits:
                    waits = list(si.on_wait)
                    keep = waits[-max_waits:]
                    for k, w in enumerate(waits[:-max_waits]):
                        nop = mybir.InstNoOp(name=f"{ins.name}-wsplit{k}")
                        nop.engine = ins.engine
                        nop.sync_info = mybir.SyncInfo(on_wait=[w], on_update=[])
                        new.append(nop)
                        nsplit += 1
                    si.on_wait = keep
                new.append(ins)
            blk.instructions = new
    return nsplit


# ---------------------------------------------------------------------------
# Device program
# ---------------------------------------------------------------------------

_CACHE = {}

_MASK2 = np.zeros((HPC, HPC * D), np.float32)
for _h in range(HPC):
    _MASK2[_h, _h * D:(_h + 1) * D] = 1.0

_PERM2 = np.array([[0.0, 1.0], [1.0, 0.0]], np.float32)


def _build_program():
    nc = bass.Bass("TRN2", num_devices=N_CORES)

    # u for this core's 2 heads, tail layout: up16[p, 128h+c] = u[2c+h, 128p+c]
    up16 = nc.dram_tensor("up16", [16, 2 * 128], F32, kind="ExternalInput")
    # wT: w_gumbel[:, jc].T  [256, 2048] bf16 (host pre-transposed)
    wTd = nc.dram_tensor("wT", [JC, L], BF16, kind="ExternalInput")
    qchunk = nc.dram_tensor("qchunk", [QC, E], F32, kind="ExternalInput")
    # kT: k[:, head cols].T  [128, 2048] bf16, rows 0:64 head0, 64:128 head1
    kTd = nc.dram_tensor("kT", [HPC * D, L], BF16, kind="ExternalInput")
    # v pre-arranged [128, NCH*128] bf16: vhp[p, 128*r + c] = v[128*r + p, c]
    vhp = nc.dram_tensor("vhp", [128, L], BF16, kind="ExternalInput")
    qfull = nc.dram_tensor("qfull", [L * H, D], F32, kind="ExternalInput")
    b16 = nc.dram_tensor("b16", [16, 2 * 128], F32, kind="ExternalInput")
    hofff = nc.dram_tensor("hofff", [HPC, 1], F32, kind="ExternalInput")
    maskin = nc.dram_tensor("maskin", [HPC, HPC * D], F32, kind="ExternalInput")
    perm2d = nc.dram_tensor("perm2", [HPC, HPC], F32, kind="ExternalInput")
    # transposed output: out[l, f] lives at outT[f, l]
    outT = nc.dram_tensor("outT", [HPC * D, L], F32, kind="ExternalOutput")
    outRows = nc.dram_tensor("outRows", [HPC, HPC * D], F32, kind="ExternalOutput")
    outIdx = nc.dram_tensor("outIdx", [HPC, 1], I32, kind="ExternalOutput")

    lpart = nc.dram_tensor("lpart", [H, L], F32)
    lrs = nc.dram_tensor("lrs", [HPC, 16, 128], F32)
    warm_in = nc.dram_tensor("warm_in", [8, 8], F32)
    warm_out = nc.dram_tensor("warm_out", [1, 8], F32)

    with TileContext(nc) as tc:
        with tc.tile_pool(name="big", bufs=1) as big, \
             tc.tile_pool(name="work", bufs=1) as work, \
             tc.tile_pool(name="ps_mm", bufs=2, space="PSUM") as ps_mm, \
             tc.tile_pool(name="ps_sm", bufs=2, space="PSUM") as ps_sm, \
             tc.tile_pool(name="ps_acc", bufs=1, space="PSUM") as ps_acc:

            # ---- prewarm the CC engine with a tiny collective ---------------
            wz = work.tile([8, 8], F32, tag="wz")
            nc.vector.memset(wz[:], 0.0)
            nc.sync.dma_start(out=warm_in[:], in_=wz[:])
            nc.gpsimd.collective_compute(
                "ReduceScatter", ALU.add,
                replica_groups=[list(range(N_CORES))],
                ins=[warm_in[:]], outs=[warm_out[:]],
            )

            # ---- sync-ring DMAs only: qt, wt (+ lpart below) ----------------
            qts = []
            for s in range(2):
                qt = big.tile([128, E], F32, tag=f"qrows{s}")
                nc.sync.dma_start(out=qt[:], in_=qchunk[s * 128:(s + 1) * 128, :])
                qts.append(qt)
            wt = [big.tile([128, L], BF16, tag=f"wt{s}", name=f"wt{s}")
                  for s in range(2)]
            for n in range(4):
                for s in range(2):
                    nc.sync.dma_start(
                        out=wt[s][:, n * 512:(n + 1) * 512],
                        in_=wTd[s * 128:(s + 1) * 128, n * 512:(n + 1) * 512],
                    )

            # ---- q_mean^T (bf16) for this j-chunk ---------------------------
            qmT = []
            for s in range(2):
                qm = work.tile([128, H], F32, tag=f"qmT{s}")
                nc.vector.reduce_sum(
                    qm[:], qts[s][:].rearrange("p (h d) -> p h d", d=D),
                    axis=mybir.AxisListType.X,
                )
                qmb = work.tile([128, H], BF16, tag=f"qmb{s}")
                nc.vector.tensor_scalar_mul(qmb[:], qm[:], 1.0 / D)
                qmT.append(qmb)

            ident = work.tile([128, 128], F32)
            make_identity(nc, ident)

            # ---- partial logits + g/8 -> lpart, then ReduceScatter ----------
            lp = work.tile([H, L], F32, tag="lp")
            for n in range(4):
                pl = ps_mm.tile([H, 512], F32, tag="mm")
                for s in range(2):
                    nc.tensor.matmul(
                        out=pl[:],
                        lhsT=qmT[s][:],
                        rhs=wt[s][:, n * 512:(n + 1) * 512],
                        start=(s == 0), stop=(s == 1),
                    )
                nc.scalar.copy(lp[:, n * 512:(n + 1) * 512], pl[:])
            nc.sync.dma_start(out=lpart[:], in_=lp[:])
            nc.gpsimd.collective_compute(
                "ReduceScatter", ALU.add,
                replica_groups=[list(range(N_CORES))],
                ins=[lpart[:]], outs=[lrs[:, :, :]],
            )

            # ---- K^T load on the tensor ring (bf16) -------------------------
            KT = big.tile([128, L], BF16, tag="KT")
            nc.gpsimd.dma_start(out=KT[:], in_=kTd[:, :])

            # ---- V load on the gpsimd ring (bf16, host pre-arranged) --------
            vtx = big.tile([128, NCH * VW], BF16, tag="vtx")
            vtx3 = vtx[:].rearrange("p (r c) -> p r c", c=VW)
            nc.vector.memset(vtx3[:, :, 128:129], 1.0)
            nc.gpsimd.dma_start(
                out=vtx3[:, :, 0:128],
                in_=vhp.rearrange("p (r c) -> p r c", c=128),
            )

            # ---- small consts -----------------------------------------------
            bt = work.tile([16, 256], F32, tag="bt")
            nc.scalar.dma_start(out=bt[:], in_=b16[:])
            upt = work.tile([16, 256], F32, tag="upt")
            nc.scalar.dma_start(out=upt[:], in_=up16[:])
            hof = work.tile([HPC, 1], F32, tag="hof")
            nc.scalar.dma_start(out=hof[:], in_=hofff[:])
            mask2 = work.tile([HPC, 128], F32, tag="mask2")
            nc.scalar.dma_start(out=mask2[:], in_=maskin[:])
            perm2 = work.tile([HPC, HPC], F32, tag="perm2")
            nc.scalar.dma_start(out=perm2[:], in_=perm2d[:])
            iot = work.tile([HPC, 16], I32, tag="iot")
            nc.gpsimd.iota(iot[:], pattern=[[1, 16]], base=0, channel_multiplier=0)
            iof = work.tile([HPC, 16], F32, tag="iof")
            nc.vector.tensor_copy(iof[:], iot[:])
            onesc = work.tile([128, 1], BF16, tag="onesc")
            nc.vector.memset(onesc[:], 1.0)
            ones12 = work.tile([1, HPC], F32, tag="ones12")
            nc.vector.memset(ones12[:], 1.0)

            # ---- v column means; broadcast fill of the output ---------------
            pvm = ps_acc.tile([1, 128], F32, tag="vm")
            for r in range(NCH):
                nc.tensor.matmul(out=pvm[:], lhsT=onesc[:],
                                 rhs=vtx3[:, r, 0:128],
                                 start=(r == 0), stop=(r == NCH - 1))
            vm0 = work.tile([1, 128], F32, tag="vm0")
            nc.vector.tensor_scalar_mul(vm0[:], pvm[:], 1.0 / L)
            pb2 = ps_sm.tile([HPC, 128], F32, tag="sm")
            nc.tensor.matmul(out=pb2[:], lhsT=ones12[:], rhs=vm0[:],
                             start=True, stop=True)
            vmb2 = work.tile([HPC, 128], F32, tag="vmb2")
            nc.vector.tensor_copy(vmb2[:], pb2[:])
            pvt = ps_sm.tile([128, 1], F32, tag="sm")
            nc.tensor.transpose(out=pvt[:], in_=vm0[:], identity=ident[0:1, 0:1])
            vmT = work.tile([128, 1], F32, tag="vmT")
            nc.vector.tensor_copy(vmT[:], pvt[:])
            vmbT = big.tile([128, L], F32, tag="vmbT")
            nc.vector.tensor_copy(vmbT[:], vmT[:].to_broadcast([128, L]))
            nc.scalar.dma_start(out=outT[:, :], in_=vmbT[:])

            # ---- gumbel + bias for this core's heads (off critical path) ----
            # bg = b - ln(-ln u);  z = lrs + bg
            s1p = work.tile([16, 256], F32, tag="s1p")
            nc.scalar.activation(s1p[:], upt[:], AF.Ln)
            s2p = work.tile([16, 256], F32, tag="s2p")
            nc.scalar.activation(s2p[:], s1p[:], AF.Ln, scale=-1.0)
            bg = work.tile([16, 256], F32, tag="bg")
            nc.vector.tensor_tensor(out=bg[:], in0=bt[:], in1=s2p[:],
                                    op=ALU.subtract)

            # ---- keep PE warm across the collective -------------------------
            for wrm in range(24):
                pw = ps_sm.tile([128, 128], F32, tag="sm", name=f"warm{wrm}")
                nc.tensor.transpose(out=pw[:], in_=ident[:], identity=ident[:])

            # ---- z = logits + g (from RS) + b; per-head argmax --------------
            # z16[p, 128h + c] = z_head_h[128p + c]
            z16 = work.tile([16, 256], F32, tag="z16")
            for h in range(2):
                nc.gpsimd.dma_start(
                    out=z16[:, 128 * h:128 * h + 128],
                    in_=lrs[h:h + 1].rearrange("o p c -> (o p) c"),
                )
            nc.vector.tensor_tensor(out=z16[:], in0=z16[:], in1=bg[:], op=ALU.add)

            mx = work.tile([16, 16], F32, tag="mx")
            idx = work.tile([16, 16], U32, tag="idx")
            for h in range(2):
                nc.vector.max_with_indices(
                    mx[:, 8 * h:8 * h + 8], idx[:, 8 * h:8 * h + 8],
                    z16[:, 128 * h:128 * h + 128],
                )
            candA = work.tile([16, 2], F32, tag="candA")
            candB = work.tile([16, 2], F32, tag="candB")
            for h in range(2):
                nc.vector.tensor_copy(candA[:, h:h + 1], mx[:, 8 * h:8 * h + 1])
                nc.vector.tensor_copy(candB[:, h:h + 1], idx[:, 8 * h:8 * h + 1])
            pA = ps_sm.tile([HPC, 16], F32, tag="sm")
            nc.tensor.transpose(out=pA[:], in_=candA[:], identity=ident[0:16, 0:16])
            pB = ps_sm.tile([HPC, 16], F32, tag="sm")
            nc.tensor.transpose(out=pB[:], in_=candB[:], identity=ident[0:16, 0:16])
            mxT = work.tile([HPC, 16], F32, tag="mxT")
            nc.vector.tensor_copy(mxT[:], pA[:])
            idxT = work.tile([HPC, 16], F32, tag="idxT")
            nc.vector.tensor_copy(idxT[:], pB[:])

            gmx = work.tile([HPC, 8], F32, tag="gmx")
            gp = work.tile([HPC, 8], U32, tag="gp")
            nc.vector.max_with_indices(gmx[:], gp[:], mxT[:])
            gpf = work.tile([HPC, 1], F32, tag="gpf")
            nc.vector.tensor_copy(gpf[:], gp[:, 0:1])
            pf = work.tile([HPC, 16], F32, tag="pf")
            nc.vector.tensor_tensor(out=pf[:], in0=iof[:],
                                    in1=gpf[:].to_broadcast([HPC, 16]),
                                    op=ALU.is_equal)
            tsel = work.tile([HPC, 16], F32, tag="tsel")
            nc.vector.tensor_tensor(out=tsel[:], in0=idxT[:], in1=pf[:],
                                    op=ALU.mult)
            inn = work.tile([HPC, 1], F32, tag="inn")
            nc.vector.reduce_sum(inn[:], tsel[:], axis=mybir.AxisListType.X)

            lself = work.tile([HPC, 1], F32, tag="lself")
            nc.vector.tensor_scalar(out=lself[:], in0=gpf[:], scalar1=128.0,
                                    scalar2=None, op0=ALU.mult)
            nc.vector.tensor_tensor(out=lself[:], in0=lself[:], in1=inn[:],
                                    op=ALU.add)
            fif = work.tile([HPC, 1], F32, tag="fif")
            nc.vector.tensor_scalar(out=fif[:], in0=lself[:], scalar1=float(H),
                                    scalar2=None, op0=ALU.mult)
            nc.vector.tensor_tensor(out=fif[:], in0=fif[:], in1=hof[:],
                                    op=ALU.add)
            lseli = work.tile([HPC, 1], I32, tag="lseli")
            nc.vector.tensor_copy(lseli[:], lself[:])
            fii = work.tile([HPC, 1], I32, tag="fii")
            nc.vector.tensor_copy(fii[:], fif[:])

            # ---- gather the two selected q rows -----------------------------
            qsel = work.tile([HPC, D], F32, tag="qsel")
            nc.gpsimd.indirect_dma_start(
                out=qsel[:], out_offset=None,
                in_=qfull[:, :],
                in_offset=bass.IndirectOffsetOnAxis(ap=fii[:, 0:1], axis=0),
            )
            nc.vector.tensor_scalar_mul(qsel[:], qsel[:], SCALE)
            # qsel2[h, 64j+k] = qsel[h, k] * (j == h): pads each head's q row
            # into its stacked-KT d-slot, via broadcast-read x maskin.
            qsel2 = work.tile([HPC, 128], F32, tag="qsel2")
            nc.vector.tensor_tensor(
                out=qsel2[:].rearrange("p (j c) -> p j c", c=D),
                in0=qsel[:].rearrange("p (o c) -> p o c", o=1)
                    .to_broadcast([HPC, 2, D]),
                in1=mask2[:].rearrange("p (j c) -> p j c", c=D),
                op=ALU.mult,
            )
            pq = ps_sm.tile([128, HPC], F32, tag="sm")
            nc.tensor.transpose(out=pq[:], in_=qsel2[:], identity=ident[0:HPC, 0:HPC])
            qT = work.tile([128, HPC], BF16, tag="qT")
            nc.vector.tensor_copy(qT[:], pq[:])

            # ---- one attention row per head (transposed scores) -------------
            psc = ps_acc.tile([128, 32], F32, tag="sc")
            for r in range(NCH):
                nc.tensor.matmul(out=psc[:, 2 * r:2 * r + 2],
                                 lhsT=KT[:, r * 128:(r + 1) * 128],
                                 rhs=qT[:], start=True, stop=True)
            escT = work.tile([128, 32], BF16, tag="escT")
            nc.scalar.activation(escT[:, 0:16], psc[:, 0:16], AF.Exp)
            nc.scalar.activation(escT[:, 16:32], psc[:, 16:32], AF.Exp)

            pav = ps_acc.tile([HPC, 129], F32, tag="av")
            for r in range(NCH):
                nc.tensor.matmul(out=pav[:], lhsT=escT[:, 2 * r:2 * r + 2],
                                 rhs=vtx3[:, r, 0:129],
                                 start=(r == 0), stop=(r == NCH - 1))
            rsum = work.tile([HPC, 1], F32, tag="rsum")
            nc.vector.reciprocal(rsum[:], pav[:, 128:129])
            att = work.tile([HPC, 128], F32, tag="att")
            nc.vector.tensor_scalar_mul(att[:], pav[:, 0:128], rsum[:, 0:1])

            # ---- rows = vmean + delta (+ cross-term if same argmax) ---------
            delta = work.tile([HPC, 128], F32, tag="delta")
            nc.vector.tensor_tensor(out=delta[:], in0=att[:], in1=vmb2[:],
                                    op=ALU.subtract)
            nc.vector.tensor_tensor(out=delta[:], in0=delta[:], in1=mask2[:],
                                    op=ALU.mult)
            psw = ps_sm.tile([HPC, 128], F32, tag="sm")
            nc.tensor.matmul(out=psw[:], lhsT=perm2[:], rhs=delta[:],
                             start=True, stop=True)
            plw = ps_sm.tile([HPC, 1], F32, tag="sm")
            nc.tensor.matmul(out=plw[:], lhsT=perm2[:], rhs=lself[:],
                             start=True, stop=True)
            eq = work.tile([HPC, 1], F32, tag="eq")
            nc.vector.tensor_tensor(out=eq[:], in0=lself[:], in1=plw[:],
                                    op=ALU.is_equal)
            cross = work.tile([HPC, 128], F32, tag="cross")
            nc.vector.tensor_tensor(out=cross[:], in0=psw[:],
                                    in1=eq[:].to_broadcast([HPC, 128]),
                                    op=ALU.mult)
            rows = work.tile([HPC, 128], F32, tag="rows")
            nc.vector.tensor_tensor(out=rows[:], in0=vmb2[:], in1=delta[:],
                                    op=ALU.add)
            nc.vector.tensor_tensor(out=rows[:], in0=rows[:], in1=cross[:],
                                    op=ALU.add)

            # ---- emit the two real rows + indices; host places them ---------
            nc.gpsimd.dma_start(out=outRows[:, :], in_=rows[:])
            nc.gpsimd.dma_start(out=outIdx[:, :], in_=lseli[:])

    _split_excess_waits(nc)
    return nc


def _make_in_maps(inputs):
    query = np.ascontiguousarray(inputs["query"], dtype=np.float32)
    key = np.ascontiguousarray(inputs["key"], dtype=np.float32)
    value = np.ascontiguousarray(inputs["value"], dtype=np.float32)
    w_gumbel = np.ascontiguousarray(inputs["w_gumbel"], dtype=np.float32)
    b_gumbel = np.ascontiguousarray(inputs["b_gumbel"], dtype=np.float32)
    gumbel_u = np.ascontiguousarray(inputs["gumbel_u"], dtype=np.float32)

    q2 = query.reshape(L, E)
    k2 = key.reshape(L, E)
    v2 = value.reshape(L, E)
    qfull = np.ascontiguousarray(query.reshape(L * H, D))
    b16v = np.ascontiguousarray(np.tile(b_gumbel.reshape(16, 128), (1, 2)))

    in_maps = []
    for c in range(N_CORES):
        cols = slice(c * HPC * D, (c + 1) * HPC * D)
        vhp = np.ascontiguousarray(
            v2[:, cols].reshape(NCH, 128, 128).transpose(1, 0, 2)
            .reshape(128, L).astype(BF16NP)
        )
        up16v = np.ascontiguousarray(
            gumbel_u[0, c * HPC:(c + 1) * HPC].reshape(2, 16, 128)
            .transpose(1, 0, 2).reshape(16, 256)
        )
        in_maps.append({
            "up16": up16v,
            "wT": np.ascontiguousarray(
                w_gumbel[:, c * JC:(c + 1) * JC].T.astype(BF16NP)),
            "qchunk": np.ascontiguousarray(q2[c * QC:(c + 1) * QC, :]),
            "kT": np.ascontiguousarray(k2[:, cols].T.astype(BF16NP)),
            "vhp": vhp,
            "qfull": qfull,
            "b16": b16v,
            "hofff": np.array([[c * HPC], [c * HPC + 1]], dtype=np.float32),
            "maskin": _MASK2,
            "perm2": _PERM2,
        })
    return in_maps


def kernel(query, key, value, w_gumbel, b_gumbel, gumbel_u):
    from concourse.bass_utils import run_bass_kernel_spmd

    if "nc" not in _CACHE:
        _CACHE["nc"] = _build_program()
    nc = _CACHE["nc"]

    in_maps = _make_in_maps({
        "query": query, "key": key, "value": value,
        "w_gumbel": w_gumbel, "b_gumbel": b_gumbel, "gumbel_u": gumbel_u,
    })
    res = run_bass_kernel_spmd(nc, in_maps, core_ids=list(range(N_CORES)))
    cores = []
    for c in range(N_CORES):
        oc = np.ascontiguousarray(res.results[c]["outT"].T)
        idx = res.results[c]["outIdx"][:, 0]
        oc[idx[0], :] = res.results[c]["outRows"][0]
        oc[idx[1], :] = res.results[c]["outRows"][1]
        cores.append(oc)
    out = np.concatenate(cores, axis=1)
    return np.ascontiguousarray(out.reshape(1, L, E))


if __name__ == "__main__":
    rng = np.random.default_rng(0)
    ins = {
        "query": rng.standard_normal((1, L, E)).astype(np.float32),
        "key": rng.standard_normal((1, L, E)).astype(np.float32),
        "value": rng.standard_normal((1, L, E)).astype(np.float32),
        "w_gumbel": (rng.standard_normal((L, L)) * 0.02).astype(np.float32),
        "b_gumbel": np.zeros(L, np.float32),
        "gumbel_u": rng.uniform(1e-6, 1 - 1e-6, (1, H, L)).astype(np.float32),
    }
    out = kernel(**ins)
    print("out", out.shape, out.dtype, np.abs(out).max())
